# revision 13
# baseline (speedup 1.0000x reference)
"""Trainium2 Bass kernel for nn_CompressedSensingConvolutional.

Problem: 200 FISTA iterations of
    re    = conv_transpose(x - conv(y_tmp, w_conv, stride 8, SAME) - b_conv,
                           w_ct, stride 8, SAME) + b_ct
    w     = y_tmp - re
    y_new = soft_threshold(w, lam)        (per-sample lam)
    y_tmp = y_new + beta_n (y_new - y_last)
with x: (64,9,9,3), output y_new: (64,72,72,3).

Structure exploited (all exact, no approximations):
  * conv_transpose with 5x5 kernel / stride 8 writes NON-overlapping 5x5
    patches at output rows/cols 8I+a, a in 0..4. Positions with row%8>4 or
    col%8>4 never receive an update, so (given b_ct=0 there) they stay 0
    bitwise forever. The live state is a 45x45x3 = [75=(a,b,ci), 9x9 blocks]
    phase-space grid of 6075 values per sample.
  * With y==0, w = -c where c = At(x - b_conv) + b_ct. If |c| <= lam
    elementwise, soft_threshold returns exactly 0 and the state never
    leaves 0: the sample's output is exactly zero. Only samples with
    max|c| > lam ("active") need the 200-iteration loop at all.
  * conv(y) restricted to the live grid is a 5x5 conv over the 9x9 block
    grid with 75 input channels -> 3 outputs = 25 shift-matmuls (K=75, M=3,
    N=81) accumulated in PSUM.

Each active sample runs on its own NeuronCore (8 cores; extra actives are
handled in additional device rounds). The 200 iterations run as a single
tc.For_i HARDWARE loop (not unrolled) with 2 iterations per trip: the NEFF
holds one copy of the ~70-instruction loop body, with the per-iteration
FISTA momentum coefficients beta_n and 1+beta_n read from small SBUF tables
indexed by the loop variable. This keeps the program ~270 instructions
instead of ~7400, which collapses NEFF load time and instruction-fetch
overhead. The 4 PSUM column-group partials of the z-phase are not summed on
DVE; instead they are copied into rows 75:87 of the combined tile and the
At-phase matmul carries 4 stacked copies of the At weights, so the group
reduction rides the PE accumulation tree instead of 3 serial DVE adds.
"""

import math
import os
import sys

import numpy as np

for _p in ("/opt/trn_rl_repo", "/root/.axon_site/_ro/trn_rl_repo"):
    if os.path.isdir(_p) and _p not in sys.path:
        sys.path.insert(0, _p)

N_ITERS = 200
N_CORES = 8
HW = 72
LOW = 9
C = 3
F = 75          # (a,b,ci): 5*5*3 live phase-space channels
NP2 = 13        # padded block grid (9 + 2 on each side)
NB = 9          # block grid
NPOS = NB * NB  # 81


def _betas(n_iters):
    """beta_n = (t_n - 1)/t_{n+1}, bit-exact fp32 mirror of the reference."""
    one, two, four = np.float32(1.0), np.float32(2.0), np.float32(4.0)
    t = np.float32(1.0)
    out = []
    for _ in range(n_iters):
        t_n = (one + np.sqrt(one + four * t * t)) / two
        out.append(np.float32((t - one) / t_n))
        t = t_n
    return out


def _beta_tables():
    """[F, N_ITERS] tables: col n = beta_n / 1+beta_n broadcast over rows."""
    b = np.asarray(_betas(N_ITERS), np.float32)
    bet = np.broadcast_to(b, (F, N_ITERS)).copy()
    bet1 = np.broadcast_to(np.float32(1.0) + b, (F, N_ITERS)).copy()
    return bet, bet1


_DEV_CACHE = {}


def _build_device(n_iters):
    """Build + compile the per-core FISTA program (SPMD, same code all cores).

    Layout: one active sample per core. State y_tmp lives in a padded
    [75, 13x13] fp32 SBUF tile (rows 0:75 of zy); rows 75:87 hold the four
    z-phase PSUM group partials, row 99 is constant 1.0. Per iteration:
      z-phase : 25 shift-matmuls (5x5 phase conv, K=75, M=3, N=81) spread
                over 4 PSUM column groups (tile_position col packing, the 4
                streams run concurrently on the PE array); +1 matmul adds bx.
      copies  : the 4 group partials go to zy rows 75:87 (2 on ACT, 2 on
                DVE, pairwise parallel) - no serial adds.
      At-phase: w = y_tmp + sum_g Wr^T P_g - b_ct as ONE K=100 matmul
                (Wcomb rows 0:75 = I, 75:87 = 4x Wr, 99 = -b_ct).
      soft    : cl = clamp(w, +-lam); y_new = w - cl   (DVE, reads PSUM)
      momentum: y_tmp' = (1+beta_n) y_new - beta_n y_last, with
                e = beta_n*y_last issued first so it reads y_last before the
                y_new overwrite (same in-order DVE queue).
    Two iterations per For_i trip (ping-pong y_new tiles) halve the
    per-trip semaphore-reset/barrier cost; odd n_iters runs the last
    iteration outside the loop. beta tables are always [75, N_ITERS] so
    every n_iters build has identical I/O.
    """
    if n_iters in _DEV_CACHE:
        return _DEV_CACHE[n_iters]

    import concourse.bacc as bacc
    import concourse.mybir as mybir
    from concourse.bass import ds
    from concourse.tile import TileContext

    f32 = mybir.dt.float32
    Alu = mybir.AluOpType

    nc = bacc.Bacc(trn_type="TRN2")
    wc_d = nc.dram_tensor("wc", [F, 75], f32, kind="ExternalInput")
    wcomb_d = nc.dram_tensor("wcomb", [100, F], f32, kind="ExternalInput")
    bx_d = nc.dram_tensor("bx", [C, NPOS], f32, kind="ExternalInput")
    i3_d = nc.dram_tensor("i3", [C, C], f32, kind="ExternalInput")
    i75_d = nc.dram_tensor("i75", [F, F], f32, kind="ExternalInput")
    onesr_d = nc.dram_tensor("onesr", [1, NPOS], f32, kind="ExternalInput")
    lam_d = nc.dram_tensor("lam2", [F, 2], f32, kind="ExternalInput")
    bet_d = nc.dram_tensor("bet", [F, N_ITERS], f32, kind="ExternalInput")
    bet1_d = nc.dram_tensor("bet1", [F, N_ITERS], f32, kind="ExternalInput")
    y_d = nc.dram_tensor("y", [F, NPOS], f32, kind="ExternalOutput")

    # shift s -> column group; g0 gets 7 shifts, g1 6 (+bx), g2/g3 6.
    grp_of = [s % 4 for s in range(25)]
    order = []  # round-robin issue order for concurrency
    by_g = [[s for s in range(25) if grp_of[s] == g] for g in range(4)]
    for r in range(7):
        for g in range(4):
            if r < len(by_g[g]):
                order.append(by_g[g][r])

    with TileContext(nc) as tc:
        with tc.tile_pool(name="const", bufs=1) as cpool, \
             tc.tile_pool(name="state", bufs=1) as spool, \
             tc.tile_pool(name="psum", bufs=1, space="PSUM") as ppool:
            wc = cpool.tile([F, 75], f32, tag="wc")
            nc.sync.dma_start(wc[:], wc_d[:])
            wcomb = cpool.tile([100, F], f32, tag="wcomb")
            nc.sync.dma_start(wcomb[:], wcomb_d[:])
            bx = cpool.tile([C, NPOS], f32, tag="bx")
            nc.sync.dma_start(bx[:], bx_d[:])
            i3 = cpool.tile([C, C], f32, tag="i3")
            nc.sync.dma_start(i3[:], i3_d[:])
            i75 = cpool.tile([F, F], f32, tag="i75")
            nc.sync.dma_start(i75[:], i75_d[:])
            lam2 = cpool.tile([F, 2], f32, tag="lam")
            nc.sync.dma_start(lam2[:], lam_d[:])
            bet = cpool.tile([F, N_ITERS], f32, tag="bet")
            nc.sync.dma_start(bet[:], bet_d[:])
            bet1 = cpool.tile([F, N_ITERS], f32, tag="bet1")
            nc.sync.dma_start(bet1[:], bet1_d[:])

            # zy rows 0:75 = y_tmp on the padded 13x13 grid.
            zy = spool.tile([F, NP2 * NP2], f32, tag="zy")
            # pp: z-group partials at rows {0,32,64,96}+0:3 (the same offsets
            # they occupy in PSUM, so one start-0 copy stages all four),
            # row 99 = constant 1.0 for the -b_ct term (outside the copy).
            pp = spool.tile([100, NPOS], f32, tag="pp")
            yna = spool.tile([F, NPOS], f32, tag="yna")
            ynb = spool.tile([F, NPOS], f32, tag="ynb")
            e = spool.tile([F, NPOS], f32, tag="e")
            cl = spool.tile([F, NPOS], f32, tag="cl")
            ft = spool.tile([F, NPOS], f32, tag="ft")
            pz = ppool.tile([128, NPOS], f32, tag="pz")
            pw = ppool.tile([F, NPOS], f32, tag="pw")

            nc.vector.memset(zy[:], 0.0)
            nc.vector.memset(pp[:], 0.0)
            nc.sync.dma_start(pp[99:100, :], onesr_d[:])
            nc.vector.memset(yna[:], 0.0)
            nc.vector.memset(ynb[:], 0.0)
            # pz rows outside the 4 written groups stay 0 forever, so the
            # wide stage copy below can sweep 32:99 unconditionally.
            nc.vector.memset(pz[:], 0.0)

            zyv = zy[:].rearrange("p (r c) -> p r c", c=NP2)
            ytv = zyv[0:F]
            yt_int = ytv[:, 2:2 + NB, 2:2 + NB]

            def body(col, ynew, ylast):
                # e = beta_n * y_last (in-order DVE: reads ylast=other tile,
                # and for the second sub-iteration the ynew written below)
                nc.vector.tensor_scalar_mul(e[:], ylast[:], bet[:, col])

                # z-phase: a~ = A_lin(y_tmp) + bx in 4 PSUM column groups
                nc.tensor.matmul(pz[32:35, :], i3[:], bx[:], start=True,
                                 stop=False, tile_position=(0, 32))
                seen = [0, 0, 0, 0]
                for s in order:
                    g = grp_of[s]
                    m, nn_ = divmod(s, 5)
                    nc.tensor.matmul(
                        pz[32 * g:32 * g + 3, :],
                        wc[:, 3 * s:3 * s + 3],
                        ytv[:, m:m + NB, nn_:nn_ + NB],
                        start=(seen[g] == 0 and g != 1),
                        stop=(seen[g] == len(by_g[g]) - 1),
                        tile_position=(0, 32 * g),
                    )
                    seen[g] += 1

                # stage the partials: ONE start-0 ACT copy sweeps all four
                # groups (rows between them are the memset zeros; row 99 of
                # pp, the ones row, is outside the copy)
                nc.scalar.copy(pp[0:99, :], pz[0:99, :])

                # At-phase: w = y_tmp + sum_g Wr^T P_g - b_ct as 2 chained
                # matmuls (identity @ y_tmp, stacked-Wr @ partials)
                nc.tensor.matmul(pw[:], i75[:], yt_int, start=True,
                                 stop=False)
                nc.tensor.matmul(pw[:], wcomb[:], pp[0:100, :], start=False,
                                 stop=True)

                # soft threshold (reads PSUM)
                nc.vector.tensor_scalar(
                    cl[:], pw[:], lam2[:, 0:1], lam2[:, 1:2], Alu.min,
                    Alu.max
                )
                nc.vector.tensor_sub(ynew[:], pw[:], cl[:])

                # momentum: y_tmp' = (1+beta)*y_new - e
                nc.vector.tensor_scalar_mul(ft[:], ynew[:], bet1[:, col])
                nc.vector.tensor_sub(yt_int, ft[:], e[:])

            n_pairs = n_iters // 2
            if n_pairs > 0:
                with tc.For_i(0, 2 * n_pairs, 2) as iv:
                    body(ds(iv, 1), yna, ynb)
                    body(ds(iv + 1, 1), ynb, yna)
            if n_iters % 2:
                body(slice(n_iters - 1, n_iters), yna, ynb)

            # final y_new: yna if n_iters is odd, ynb if even (zeros if 0)
            nc.sync.dma_start(y_d[:], (yna if n_iters % 2 else ynb)[:])

    nc.compile()
    _DEV_CACHE[n_iters] = nc
    return nc


def kernel(x, lam, w_conv, b_conv, w_ct, b_ct):
    from concourse import bass_utils

    x = np.asarray(x, np.float32)
    lam = np.asarray(lam, np.float32)
    w_conv = np.asarray(w_conv, np.float32)
    b_conv = np.asarray(b_conv, np.float32)
    w_ct = np.asarray(w_ct, np.float32)
    b_ct = np.asarray(b_ct, np.float32)
    B = x.shape[0]

    # ---- host analysis (exact): c = At(x - b_conv) + b_ct on the live grid
    w_rev = w_ct[::-1, ::-1]                      # [a,b,ci,co] = w_ct[4-a,4-b,ci,co]
    xb = x - b_conv                               # (B,9,9,3)
    # c[s, a, b, co, I, J]
    c = np.einsum('abeo,sije->sabo' 'ij', w_rev, xb, optimize=True)
    c = c + b_ct[None, None, None, :, None, None]
    cmax = np.abs(c).max(axis=(1, 2, 3, 4, 5))
    active = cmax > lam * np.float32(1.0 - 1e-5)
    act_idx = np.where(active)[0]

    # ---- device weights (same for every core)
    aa, bb_, cc = np.meshgrid(np.arange(5), np.arange(5), np.arange(C), indexing='ij')
    # Wc_all[f=(a,b,ci), 3*s+co] = w_conv[8m+a, 8n+b, ci, co],  s = 5m+n
    Wc_all = np.zeros((F, 75), np.float32)
    for s in range(25):
        m, n = divmod(s, 5)
        blk = w_conv[8 * m + aa, 8 * n + bb_, cc, :]      # (5,5,3,3)
        Wc_all[:, 3 * s:3 * s + 3] = blk.reshape(F, C)
    # Wcomb: rows {0,32,64,96}+0:3 = Wr (At weights, [ci, (a,b,co)] =
    # w_rev[a,b,ci,co]) matching the 4 z-group partial rows of pp;
    # row 99 = -b_ct (ones row in pp); everything else 0.
    Wcomb = np.zeros((100, F), np.float32)
    Wr = np.transpose(w_rev, (2, 0, 1, 3)).reshape(C, F)
    for g in range(4):
        Wcomb[32 * g:32 * g + 3, :] = Wr
    Wcomb[99, :] = np.broadcast_to(-b_ct, (5, 5, C)).reshape(F)
    I3 = np.eye(C, dtype=np.float32)
    I75 = np.eye(F, dtype=np.float32)
    OnesR = np.ones((1, NPOS), np.float32)
    BetT, Bet1T = _beta_tables()

    out = np.zeros((B, HW, HW, C), np.float32)

    # Non-patch positions evolve autonomously: w = y - b_ct per channel.
    # Exact when b_ct == 0 (it is, per the model); otherwise computed here.
    if np.any(b_ct != 0.0):
        betas = _betas(N_ITERS)
        yv = np.zeros((B, C), np.float32)
        yl = np.zeros((B, C), np.float32)
        for n in range(N_ITERS):
            w_np = yv - b_ct[None, :]
            y_new = (np.maximum(w_np - lam[:, None], 0)
                     - np.maximum(-w_np - lam[:, None], 0)).astype(np.float32)
            yv = y_new + np.float32(betas[n]) * (y_new - yl)
            yl = y_new
        mask = np.ones((HW, HW), bool)
        rows = (np.arange(HW) % 8) < 5
        mask[np.ix_(rows, rows)] = False          # live-grid positions
        out[:, mask, :] = yl[:, None, :]

    nc = _build_device(N_ITERS)

    n_rounds = max(1, math.ceil(len(act_idx) / N_CORES))
    zero_bx = np.zeros((C, NPOS), np.float32)
    one_lam = np.stack([np.ones(F, np.float32), -np.ones(F, np.float32)], axis=1)
    for r in range(n_rounds):
        batch = act_idx[r * N_CORES:(r + 1) * N_CORES]
        in_maps = []
        for k in range(N_CORES):
            if k < len(batch):
                s = int(batch[k])
                bx = np.ascontiguousarray(
                    (b_conv[:, None] - x[s].reshape(NPOS, C).T).astype(np.float32))
                lam2 = np.stack([np.full(F, lam[s], np.float32),
                                 np.full(F, -lam[s], np.float32)], axis=1)
            else:
                bx, lam2 = zero_bx, one_lam
            in_maps.append({
                "wc": Wc_all, "wcomb": Wcomb, "bx": bx, "i3": I3,
                "i75": I75, "onesr": OnesR,
                "lam2": np.ascontiguousarray(lam2),
                "bet": BetT, "bet1": Bet1T,
            })
        res = bass_utils.run_bass_kernel_spmd(nc, in_maps, core_ids=list(range(N_CORES)))
        for k in range(len(batch)):
            s = int(batch[k])
            ya = res.results[k]["y"].reshape(5, 5, C, NB, NB)
            # out[s, 8I+a, 8J+b, ci] = ya[a,b,ci,I,J]
            blk = np.transpose(ya, (3, 0, 4, 1, 2))   # (I,a,J,b,ci)
            ov = out[s].reshape(NB, 8, NB, 8, C)
            ov[:, :5, :, :5, :] = blk
    return out


# revision 17
# speedup vs baseline: 22.7463x; 22.7463x over previous
"""Trainium2 Bass kernel for nn_CompressedSensingConvolutional.

Problem: 200 FISTA iterations of
    re    = conv_transpose(x - conv(y_tmp, w_conv, stride 8, SAME) - b_conv,
                           w_ct, stride 8, SAME) + b_ct
    w     = y_tmp - re
    y_new = soft_threshold(w, lam)        (per-sample lam)
    y_tmp = y_new + beta_n (y_new - y_last)
with x: (64,9,9,3), output y_new: (64,72,72,3).

Structure exploited (all exact, no approximations):
  * conv_transpose with 5x5 kernel / stride 8 writes NON-overlapping 5x5
    patches at output rows/cols 8I+a, a in 0..4. Positions with row%8>4 or
    col%8>4 never receive an update, so (given b_ct=0 there) they stay 0
    bitwise forever. The live state is a 45x45x3 = [75=(a,b,ci), 9x9 blocks]
    phase-space grid of 6075 values per sample.
  * With y==0, w = -c where c = At(x - b_conv) + b_ct. If |c| <= lam
    elementwise, soft_threshold returns exactly 0 and the state never
    leaves 0: the sample's output is exactly zero. Only samples with
    max|c| > lam ("active") need the 200-iteration loop at all.
  * conv(y) restricted to the live grid is a 5x5 conv over the 9x9 block
    grid with 75 input channels -> 3 outputs = 25 shift-matmuls (K=75, M=3,
    N=81) accumulated in PSUM.

Each active sample runs on its own NeuronCore (8 cores; extra actives are
handled in additional device rounds). The 200 iterations run as a single
tc.For_i HARDWARE loop (not unrolled) with 2 iterations per trip: the NEFF
holds one copy of the ~70-instruction loop body, with the per-iteration
FISTA momentum coefficients beta_n and 1+beta_n read from small SBUF tables
indexed by the loop variable. This keeps the program ~270 instructions
instead of ~7400, which collapses NEFF load time and instruction-fetch
overhead. The 4 PSUM column-group partials of the z-phase are not summed on
DVE; instead they are copied into rows 75:87 of the combined tile and the
At-phase matmul carries 4 stacked copies of the At weights, so the group
reduction rides the PE accumulation tree instead of 3 serial DVE adds.
"""

import math
import os
import sys

import numpy as np

for _p in ("/opt/trn_rl_repo", "/root/.axon_site/_ro/trn_rl_repo"):
    if os.path.isdir(_p) and _p not in sys.path:
        sys.path.insert(0, _p)

N_ITERS = 200
N_CORES = 8
UNROLL = 4      # FISTA iterations per For_i trip (1 = single yn tile)
HW = 72
LOW = 9
C = 3
F = 75          # (a,b,ci): 5*5*3 live phase-space channels
NP2 = 13        # padded block grid (9 + 2 on each side)
NB = 9          # block grid
NPOS = NB * NB  # 81


def _betas(n_iters):
    """beta_n = (t_n - 1)/t_{n+1}, bit-exact fp32 mirror of the reference."""
    one, two, four = np.float32(1.0), np.float32(2.0), np.float32(4.0)
    t = np.float32(1.0)
    out = []
    for _ in range(n_iters):
        t_n = (one + np.sqrt(one + four * t * t)) / two
        out.append(np.float32((t - one) / t_n))
        t = t_n
    return out


def _beta_tables():
    """[F, N_ITERS] tables: col n = beta_n / 1+beta_n broadcast over rows."""
    b = np.asarray(_betas(N_ITERS), np.float32)
    bet = np.broadcast_to(b, (F, N_ITERS)).copy()
    bet1 = np.broadcast_to(np.float32(1.0) + b, (F, N_ITERS)).copy()
    return bet, bet1


_DEV_CACHE = {}


def _build_device(n_iters):
    """Build + compile the per-core FISTA program (SPMD, same code all cores).

    Layout: one active sample per core. State y_tmp lives in a padded
    [75, 13x13] fp32 SBUF tile (rows 0:75 of zy); rows 75:87 hold the four
    z-phase PSUM group partials, row 99 is constant 1.0. Per iteration:
      z-phase : 25 shift-matmuls (5x5 phase conv, K=75, M=3, N=81) spread
                over 4 PSUM column groups (tile_position col packing, the 4
                streams run concurrently on the PE array); +1 matmul adds bx.
      copies  : the 4 group partials go to zy rows 75:87 (2 on ACT, 2 on
                DVE, pairwise parallel) - no serial adds.
      At-phase: w = y_tmp + sum_g Wr^T P_g - b_ct as ONE K=100 matmul
                (Wcomb rows 0:75 = I, 75:87 = 4x Wr, 99 = -b_ct).
      soft    : cl = clamp(w, +-lam); y_new = w - cl   (DVE, reads PSUM)
      momentum: y_tmp' = (1+beta_n) y_new - beta_n y_last, with
                e = beta_n*y_last issued first so it reads y_last before the
                y_new overwrite (same in-order DVE queue).
    Two iterations per For_i trip (ping-pong y_new tiles) halve the
    per-trip semaphore-reset/barrier cost; odd n_iters runs the last
    iteration outside the loop. beta tables are always [75, N_ITERS] so
    every n_iters build has identical I/O.
    """
    key = (n_iters, UNROLL)
    if key in _DEV_CACHE:
        return _DEV_CACHE[key]

    import concourse.bacc as bacc
    import concourse.mybir as mybir
    from concourse.bass import ds
    from concourse.tile import TileContext

    f32 = mybir.dt.float32
    Alu = mybir.AluOpType

    nc = bacc.Bacc(trn_type="TRN2")
    wc_d = nc.dram_tensor("wc", [F, 75], f32, kind="ExternalInput")
    wcomb_d = nc.dram_tensor("wcomb", [100, F], f32, kind="ExternalInput")
    bx_d = nc.dram_tensor("bx", [C, NPOS], f32, kind="ExternalInput")
    i3_d = nc.dram_tensor("i3", [C, C], f32, kind="ExternalInput")
    i75_d = nc.dram_tensor("i75", [F, F], f32, kind="ExternalInput")
    onesr_d = nc.dram_tensor("onesr", [1, NPOS], f32, kind="ExternalInput")
    lam_d = nc.dram_tensor("lam2", [F, 2], f32, kind="ExternalInput")
    bet_d = nc.dram_tensor("bet", [F, N_ITERS], f32, kind="ExternalInput")
    bet1_d = nc.dram_tensor("bet1", [F, N_ITERS], f32, kind="ExternalInput")
    y_d = nc.dram_tensor("y", [F, NPOS], f32, kind="ExternalOutput")

    # shift s -> column group; g0 gets 7 shifts, g1 6 (+bx), g2/g3 6.
    grp_of = [s % 4 for s in range(25)]
    order = []  # round-robin issue order for concurrency
    by_g = [[s for s in range(25) if grp_of[s] == g] for g in range(4)]
    for r in range(7):
        for g in range(4):
            if r < len(by_g[g]):
                order.append(by_g[g][r])

    with TileContext(nc) as tc:
        with tc.tile_pool(name="const", bufs=1) as cpool, \
             tc.tile_pool(name="state", bufs=1) as spool, \
             tc.tile_pool(name="psum", bufs=1, space="PSUM") as ppool:
            wc = cpool.tile([F, 75], f32, tag="wc")
            nc.sync.dma_start(wc[:], wc_d[:])
            wcomb = cpool.tile([100, F], f32, tag="wcomb")
            nc.sync.dma_start(wcomb[:], wcomb_d[:])
            bx = cpool.tile([C, NPOS], f32, tag="bx")
            nc.sync.dma_start(bx[:], bx_d[:])
            i3 = cpool.tile([C, C], f32, tag="i3")
            nc.sync.dma_start(i3[:], i3_d[:])
            i75 = cpool.tile([F, F], f32, tag="i75")
            nc.sync.dma_start(i75[:], i75_d[:])
            lam2 = cpool.tile([F, 2], f32, tag="lam")
            nc.sync.dma_start(lam2[:], lam_d[:])
            bet = cpool.tile([F, N_ITERS], f32, tag="bet")
            nc.sync.dma_start(bet[:], bet_d[:])
            bet1 = cpool.tile([F, N_ITERS], f32, tag="bet1")
            nc.sync.dma_start(bet1[:], bet1_d[:])

            # zy rows 0:75 = y_tmp on the padded 13x13 grid.
            zy = spool.tile([F, NP2 * NP2], f32, tag="zy")
            # pp: z-group partials at rows {0,32,64,96}+0:3 (the same offsets
            # they occupy in PSUM, so one start-0 copy stages all four),
            # row 99 = constant 1.0 for the -b_ct term (outside the copy).
            pp = spool.tile([100, NPOS], f32, tag="pp")
            yna = spool.tile([F, NPOS], f32, tag="yna")
            ynb = spool.tile([F, NPOS], f32, tag="ynb")
            e = spool.tile([F, NPOS], f32, tag="e")
            cl = spool.tile([F, NPOS], f32, tag="cl")
            ft = spool.tile([F, NPOS], f32, tag="ft")
            pz = ppool.tile([128, NPOS], f32, tag="pz")
            pw = ppool.tile([F, NPOS], f32, tag="pw")

            nc.vector.memset(zy[:], 0.0)
            nc.vector.memset(pp[:], 0.0)
            nc.sync.dma_start(pp[99:100, :], onesr_d[:])
            nc.vector.memset(yna[:], 0.0)
            nc.vector.memset(ynb[:], 0.0)
            # pz rows outside the 4 written groups stay 0 forever, so the
            # wide stage copy below can sweep 32:99 unconditionally.
            nc.vector.memset(pz[:], 0.0)

            zyv = zy[:].rearrange("p (r c) -> p r c", c=NP2)
            ytv = zyv[0:F]
            yt_int = ytv[:, 2:2 + NB, 2:2 + NB]

            def body(col, ynew, ylast):
                # e = beta_n * y_last (in-order DVE: reads ylast=other tile,
                # and for the second sub-iteration the ynew written below)
                nc.vector.tensor_scalar_mul(e[:], ylast[:], bet[:, col])

                # z-phase: a~ = A_lin(y_tmp) + bx in 4 PSUM column groups
                nc.tensor.matmul(pz[32:35, :], i3[:], bx[:], start=True,
                                 stop=False, tile_position=(0, 32))
                seen = [0, 0, 0, 0]
                for s in order:
                    g = grp_of[s]
                    m, nn_ = divmod(s, 5)
                    nc.tensor.matmul(
                        pz[32 * g:32 * g + 3, :],
                        wc[:, 3 * s:3 * s + 3],
                        ytv[:, m:m + NB, nn_:nn_ + NB],
                        start=(seen[g] == 0 and g != 1),
                        stop=(seen[g] == len(by_g[g]) - 1),
                        tile_position=(0, 32 * g),
                    )
                    seen[g] += 1

                # stage the partials: ONE start-0 ACT copy sweeps all four
                # groups (rows between them are the memset zeros; row 99 of
                # pp, the ones row, is outside the copy)
                nc.scalar.copy(pp[0:99, :], pz[0:99, :])

                # At-phase: w = y_tmp + sum_g Wr^T P_g - b_ct as 2 chained
                # matmuls (identity @ y_tmp, stacked-Wr @ partials)
                nc.tensor.matmul(pw[:], i75[:], yt_int, start=True,
                                 stop=False)
                nc.tensor.matmul(pw[:], wcomb[:], pp[0:100, :], start=False,
                                 stop=True)

                # soft threshold (reads PSUM)
                nc.vector.tensor_scalar(
                    cl[:], pw[:], lam2[:, 0:1], lam2[:, 1:2], Alu.min,
                    Alu.max
                )
                nc.vector.tensor_sub(ynew[:], pw[:], cl[:])

                # momentum: y_tmp' = (1+beta)*y_new - e
                nc.vector.tensor_scalar_mul(ft[:], ynew[:], bet1[:, col])
                nc.vector.tensor_sub(yt_int, ft[:], e[:])

            def tiles_of(n):
                # iteration n writes yna when n is even (UNROLL=1: single
                # tile, e reads the old value first on the in-order DVE)
                if UNROLL == 1:
                    return yna, yna
                return (yna, ynb) if n % 2 == 0 else (ynb, yna)

            n_trips = n_iters // UNROLL
            if n_trips > 0:
                with tc.For_i(0, UNROLL * n_trips, UNROLL) as iv:
                    for u in range(UNROLL):
                        yn_new, yn_last = tiles_of(u)
                        body(ds(iv + u, 1) if u else ds(iv, 1),
                             yn_new, yn_last)
            for k in range(UNROLL * n_trips, n_iters):
                yn_new, yn_last = tiles_of(k)
                body(slice(k, k + 1), yn_new, yn_last)

            # final y_new tile: parity of the last iteration
            fin = yna if (UNROLL == 1 or n_iters == 0 or (n_iters - 1) % 2 == 0) \
                else ynb
            nc.sync.dma_start(y_d[:], fin[:])

    nc.compile()
    _DEV_CACHE[n_iters] = nc
    return nc


def kernel(x, lam, w_conv, b_conv, w_ct, b_ct):
    from concourse import bass_utils

    x = np.asarray(x, np.float32)
    lam = np.asarray(lam, np.float32)
    w_conv = np.asarray(w_conv, np.float32)
    b_conv = np.asarray(b_conv, np.float32)
    w_ct = np.asarray(w_ct, np.float32)
    b_ct = np.asarray(b_ct, np.float32)
    B = x.shape[0]

    # ---- host analysis (exact): c = At(x - b_conv) + b_ct on the live grid
    w_rev = w_ct[::-1, ::-1]                      # [a,b,ci,co] = w_ct[4-a,4-b,ci,co]
    xb = x - b_conv                               # (B,9,9,3)
    # c[s, a, b, co, I, J]
    c = np.einsum('abeo,sije->sabo' 'ij', w_rev, xb, optimize=True)
    c = c + b_ct[None, None, None, :, None, None]
    cmax = np.abs(c).max(axis=(1, 2, 3, 4, 5))
    active = cmax > lam * np.float32(1.0 - 1e-5)
    act_idx = np.where(active)[0]

    # ---- device weights (same for every core)
    aa, bb_, cc = np.meshgrid(np.arange(5), np.arange(5), np.arange(C), indexing='ij')
    # Wc_all[f=(a,b,ci), 3*s+co] = w_conv[8m+a, 8n+b, ci, co],  s = 5m+n
    Wc_all = np.zeros((F, 75), np.float32)
    for s in range(25):
        m, n = divmod(s, 5)
        blk = w_conv[8 * m + aa, 8 * n + bb_, cc, :]      # (5,5,3,3)
        Wc_all[:, 3 * s:3 * s + 3] = blk.reshape(F, C)
    # Wcomb: rows {0,32,64,96}+0:3 = Wr (At weights, [ci, (a,b,co)] =
    # w_rev[a,b,ci,co]) matching the 4 z-group partial rows of pp;
    # row 99 = -b_ct (ones row in pp); everything else 0.
    Wcomb = np.zeros((100, F), np.float32)
    Wr = np.transpose(w_rev, (2, 0, 1, 3)).reshape(C, F)
    for g in range(4):
        Wcomb[32 * g:32 * g + 3, :] = Wr
    Wcomb[99, :] = np.broadcast_to(-b_ct, (5, 5, C)).reshape(F)
    I3 = np.eye(C, dtype=np.float32)
    I75 = np.eye(F, dtype=np.float32)
    OnesR = np.ones((1, NPOS), np.float32)
    BetT, Bet1T = _beta_tables()

    out = np.zeros((B, HW, HW, C), np.float32)

    # Non-patch positions evolve autonomously: w = y - b_ct per channel.
    # Exact when b_ct == 0 (it is, per the model); otherwise computed here.
    if np.any(b_ct != 0.0):
        betas = _betas(N_ITERS)
        yv = np.zeros((B, C), np.float32)
        yl = np.zeros((B, C), np.float32)
        for n in range(N_ITERS):
            w_np = yv - b_ct[None, :]
            y_new = (np.maximum(w_np - lam[:, None], 0)
                     - np.maximum(-w_np - lam[:, None], 0)).astype(np.float32)
            yv = y_new + np.float32(betas[n]) * (y_new - yl)
            yl = y_new
        mask = np.ones((HW, HW), bool)
        rows = (np.arange(HW) % 8) < 5
        mask[np.ix_(rows, rows)] = False          # live-grid positions
        out[:, mask, :] = yl[:, None, :]

    nc = _build_device(N_ITERS)

    n_rounds = max(1, math.ceil(len(act_idx) / N_CORES))
    zero_bx = np.zeros((C, NPOS), np.float32)
    one_lam = np.stack([np.ones(F, np.float32), -np.ones(F, np.float32)], axis=1)
    for r in range(n_rounds):
        batch = act_idx[r * N_CORES:(r + 1) * N_CORES]
        in_maps = []
        for k in range(N_CORES):
            if k < len(batch):
                s = int(batch[k])
                bx = np.ascontiguousarray(
                    (b_conv[:, None] - x[s].reshape(NPOS, C).T).astype(np.float32))
                lam2 = np.stack([np.full(F, lam[s], np.float32),
                                 np.full(F, -lam[s], np.float32)], axis=1)
            else:
                bx, lam2 = zero_bx, one_lam
            in_maps.append({
                "wc": Wc_all, "wcomb": Wcomb, "bx": bx, "i3": I3,
                "i75": I75, "onesr": OnesR,
                "lam2": np.ascontiguousarray(lam2),
                "bet": BetT, "bet1": Bet1T,
            })
        res = bass_utils.run_bass_kernel_spmd(nc, in_maps, core_ids=list(range(N_CORES)))
        for k in range(len(batch)):
            s = int(batch[k])
            ya = res.results[k]["y"].reshape(5, 5, C, NB, NB)
            # out[s, 8I+a, 8J+b, ci] = ya[a,b,ci,I,J]
            blk = np.transpose(ya, (3, 0, 4, 1, 2))   # (I,a,J,b,ci)
            ov = out[s].reshape(NB, 8, NB, 8, C)
            ov[:, :5, :, :5, :] = blk
    return out
